# revision 5
# baseline (speedup 1.0000x reference)
"""Trainium2 Bass kernel for CropConv (stride-2 3x3 conv, B=32 CIN=COUT=256,
H=W=64 -> 32x32; the crop mask is provably all-ones so output == conv output).

Strategy: data-parallel over batch across 8 NeuronCores (4 images/core).
Host pads each image to 66x66 (zero top row / left col; bottom/right pad
never read), so per core the conv is exactly 18 uniform accumulated matmuls
per 512-position output tile: 9 kernel taps x 2 cin-128-chunks, contracting
cin on the PE partition dim. The moving operand is a stride-2 window into
the SBUF-resident image (no im2col materialization). PSUM accumulates fp32;
operands use float32r, which streams at 1 row/cycle for free dims >= 256.
"""

import numpy as np

import concourse.bacc as bacc
import concourse.mybir as mybir
import concourse.tile as tile
from concourse.bass_utils import run_bass_kernel_spmd

B, CIN, COUT, H, W = 32, 256, 256, 64, 64
HP = H + 2  # padded
OH, OW = 32, 32
NCORES = 8
BL = B // NCORES          # images per core
KC = CIN // 128           # cin chunks
MC = COUT // 128          # cout chunks
NT = 2                    # output row-halves per image (16 rows x 32 cols = 512)
ROWS_PER_TILE = OH // NT

TAPS = [(ky, kx) for ky in range(3) for kx in range(3)]

_CACHE = {}


def _build(mm_dtype="f32r"):
    io_dt = {
        "bf16": mybir.dt.bfloat16,
        "f32r": mybir.dt.float32r,
        "f32": mybir.dt.float32,
    }[mm_dtype]
    nc = bacc.Bacc("TRN2", target_bir_lowering=False, debug=False, num_devices=NCORES)
    x = nc.dram_tensor("x", [BL, KC, 128, HP * HP], io_dt, kind="ExternalInput")
    w = nc.dram_tensor("w", [128, 9 * KC * COUT], io_dt, kind="ExternalInput")
    y = nc.dram_tensor("y", [BL, MC, 128, OH * OW], mybir.dt.float32,
                       kind="ExternalOutput")

    with tile.TileContext(nc) as tc:
        with (
            tc.tile_pool(name="wpool", bufs=1) as wpool,
            tc.tile_pool(name="xpool", bufs=BL * KC) as xpool,
            tc.tile_pool(name="opool", bufs=4) as opool,
            tc.tile_pool(name="psum", bufs=8, space="PSUM") as psum_pool,
        ):
            w_sb = wpool.tile([128, 9 * KC * COUT], io_dt)
            nc.sync.dma_start(w_sb[:], w.ap()[:])

            x_sb = {}
            for b in range(BL):
                for kc in range(KC):
                    t = xpool.tile([128, HP * HP], io_dt, tag="ximg")
                    nc.sync.dma_start(t[:], x.ap()[b, kc])
                    x_sb[(b, kc)] = t

            for b in range(BL):
                for mc in range(MC):
                    for nt in range(NT):
                        ps = psum_pool.tile([128, ROWS_PER_TILE * OW],
                                            mybir.dt.float32)
                        n_mm = KC * len(TAPS)
                        i_mm = 0
                        for kc in range(KC):
                            xv = x_sb[(b, kc)][:].rearrange(
                                "p (h w) -> p h w", h=HP)
                            for (ky, kx) in TAPS:
                                # padded input row for output row oy is
                                # 2*oy + ky; col for ox is 2*ox + kx
                                r0 = 2 * nt * ROWS_PER_TILE + ky
                                rhs = xv[:, r0:r0 + 2 * ROWS_PER_TILE:2,
                                         kx:kx + 2 * OW:2]
                                lhsT = w_sb[:, ((ky * 3 + kx) * KC + kc) * COUT
                                            + mc * 128:][:, :128]
                                nc.tensor.matmul(
                                    ps[:], lhsT, rhs,
                                    start=(i_mm == 0), stop=(i_mm == n_mm - 1),
                                )
                                i_mm += 1
                        ot = opool.tile([128, ROWS_PER_TILE * OW],
                                        mybir.dt.float32, tag="ostage")
                        nc.vector.tensor_copy(ot[:], ps[:])
                        nc.sync.dma_start(
                            y.ap()[b, mc, :,
                                   nt * ROWS_PER_TILE * OW:
                                   (nt + 1) * ROWS_PER_TILE * OW],
                            ot[:],
                        )
    nc.compile()
    return nc


def _get(mm_dtype="f32r"):
    if mm_dtype not in _CACHE:
        _CACHE[mm_dtype] = _build(mm_dtype)
    return _CACHE[mm_dtype]


def _prep_inputs(x, weight, mm_dtype="f32r"):
    np_dt = np.float32
    if mm_dtype == "bf16":
        import ml_dtypes
        np_dt = ml_dtypes.bfloat16
    # x: [B, CIN, H, W] -> pad to [B, CIN, HP, HP] (top row / left col zero)
    xf = np.asarray(x, dtype=np.float32)
    xp = np.zeros((B, CIN, HP, HP), dtype=np_dt)
    xp[:, :, 1:1 + H, 1:1 + W] = xf
    xs = np.ascontiguousarray(
        xp.reshape(NCORES, BL, KC, 128, HP * HP))
    # weight: [COUT, CIN, 3, 3] -> [p, tap, kc, cout] -> [128, 9*KC*COUT]
    wh = np.asarray(weight, dtype=np.float32).transpose(2, 3, 1, 0)  # ky,kx,cin,cout
    wh = wh.reshape(9, KC, 128, COUT).transpose(2, 0, 1, 3)
    wh = np.ascontiguousarray(wh.reshape(128, 9 * KC * COUT)).astype(np_dt)
    return [{"x": xs[c], "w": wh} for c in range(NCORES)]


def run(x, weight, mm_dtype="f32r", **spmd_kwargs):
    nc = _get(mm_dtype)
    in_maps = _prep_inputs(x, weight, mm_dtype)
    res = run_bass_kernel_spmd(nc, in_maps, core_ids=list(range(NCORES)),
                               **spmd_kwargs)
    out = np.empty((B, COUT, OH, OW), dtype=np.float32)
    for c in range(NCORES):
        out[c * BL:(c + 1) * BL] = res.results[c]["y"].reshape(BL, COUT, OH, OW)
    return out, res


def kernel(x, weight):
    out, _ = run(x, weight)
    return out


# revision 7
# speedup vs baseline: 1.0127x; 1.0127x over previous
"""Trainium2 Bass kernel for CropConv (stride-2 3x3 conv, B=32 CIN=COUT=256,
H=W=64 -> 32x32; the crop mask is provably all-ones so output == conv output).

Strategy: data-parallel over batch across 8 NeuronCores (4 images/core).
Host pads each image to 66x66 (zero top row / left col; bottom/right pad
never read), so per core the conv is exactly 18 uniform accumulated matmuls
per 512-position output tile: 9 kernel taps x 2 cin-128-chunks, contracting
cin on the PE partition dim. The moving operand is a stride-2 window into
the SBUF-resident image (no im2col materialization). PSUM accumulates fp32;
operands use float32r, which streams at 1 row/cycle for free dims >= 256.
"""

import numpy as np

import concourse.bacc as bacc
import concourse.mybir as mybir
import concourse.tile as tile
from concourse.bass_utils import run_bass_kernel_spmd

B, CIN, COUT, H, W = 32, 256, 256, 64, 64
HP = H + 2  # padded
OH, OW = 32, 32
NCORES = 8
BL = B // NCORES          # images per core
KC = CIN // 128           # cin chunks
MC = COUT // 128          # cout chunks
NT = 2                    # output row-halves per image (16 rows x 32 cols = 512)
ROWS_PER_TILE = OH // NT

TAPS = [(ky, kx) for ky in range(3) for kx in range(3)]

_CACHE = {}


def _build(mm_dtype="f32r"):
    io_dt = {
        "bf16": mybir.dt.bfloat16,
        "f16": mybir.dt.float16,
        "f32r": mybir.dt.float32r,
        "f32": mybir.dt.float32,
    }[mm_dtype]
    nc = bacc.Bacc("TRN2", target_bir_lowering=False, debug=False, num_devices=NCORES)
    x = nc.dram_tensor("x", [BL, KC, 128, HP * HP], io_dt, kind="ExternalInput")
    w = nc.dram_tensor("w", [128, 9 * KC * COUT], io_dt, kind="ExternalInput")
    y = nc.dram_tensor("y", [BL, MC, 128, OH * OW], mybir.dt.float32,
                       kind="ExternalOutput")

    with tile.TileContext(nc) as tc:
        with (
            tc.tile_pool(name="wpool", bufs=1) as wpool,
            tc.tile_pool(name="xpool", bufs=BL * KC) as xpool,
            tc.tile_pool(name="opool", bufs=4) as opool,
            tc.tile_pool(name="psum", bufs=8, space="PSUM") as psum_pool,
        ):
            w_sb = wpool.tile([128, 9 * KC * COUT], io_dt)
            nc.sync.dma_start(w_sb[:], w.ap()[:])

            x_sb = {}
            for b in range(BL):
                for kc in range(KC):
                    t = xpool.tile([128, HP * HP], io_dt, tag="ximg")
                    nc.sync.dma_start(t[:], x.ap()[b, kc])
                    x_sb[(b, kc)] = t

            for b in range(BL):
                for mc in range(MC):
                    for nt in range(NT):
                        ps = psum_pool.tile([128, ROWS_PER_TILE * OW],
                                            mybir.dt.float32)
                        n_mm = KC * len(TAPS)
                        i_mm = 0
                        for kc in range(KC):
                            xv = x_sb[(b, kc)][:].rearrange(
                                "p (h w) -> p h w", h=HP)
                            for (ky, kx) in TAPS:
                                # padded input row for output row oy is
                                # 2*oy + ky; col for ox is 2*ox + kx
                                r0 = 2 * nt * ROWS_PER_TILE + ky
                                rhs = xv[:, r0:r0 + 2 * ROWS_PER_TILE:2,
                                         kx:kx + 2 * OW:2]
                                lhsT = w_sb[:, ((ky * 3 + kx) * KC + kc) * COUT
                                            + mc * 128:][:, :128]
                                nc.tensor.matmul(
                                    ps[:], lhsT, rhs,
                                    start=(i_mm == 0), stop=(i_mm == n_mm - 1),
                                )
                                i_mm += 1
                        ot = opool.tile([128, ROWS_PER_TILE * OW],
                                        mybir.dt.float32, tag="ostage")
                        nc.vector.tensor_copy(ot[:], ps[:])
                        nc.sync.dma_start(
                            y.ap()[b, mc, :,
                                   nt * ROWS_PER_TILE * OW:
                                   (nt + 1) * ROWS_PER_TILE * OW],
                            ot[:],
                        )
    nc.compile()
    return nc


def _get(mm_dtype="f32r"):
    if mm_dtype not in _CACHE:
        _CACHE[mm_dtype] = _build(mm_dtype)
    return _CACHE[mm_dtype]


def _prep_inputs(x, weight, mm_dtype="f32r"):
    np_dt = np.float32
    if mm_dtype == "bf16":
        import ml_dtypes
        np_dt = ml_dtypes.bfloat16
    elif mm_dtype == "f16":
        np_dt = np.float16
    # x: [B, CIN, H, W] -> pad to [B, CIN, HP, HP] (top row / left col zero)
    xf = np.asarray(x, dtype=np.float32)
    xp = np.zeros((B, CIN, HP, HP), dtype=np_dt)
    xp[:, :, 1:1 + H, 1:1 + W] = xf
    xs = np.ascontiguousarray(
        xp.reshape(NCORES, BL, KC, 128, HP * HP))
    # weight: [COUT, CIN, 3, 3] -> [p, tap, kc, cout] -> [128, 9*KC*COUT]
    wh = np.asarray(weight, dtype=np.float32).transpose(2, 3, 1, 0)  # ky,kx,cin,cout
    wh = wh.reshape(9, KC, 128, COUT).transpose(2, 0, 1, 3)
    wh = np.ascontiguousarray(wh.reshape(128, 9 * KC * COUT)).astype(np_dt)
    return [{"x": xs[c], "w": wh} for c in range(NCORES)]


def run(x, weight, mm_dtype="f32r", **spmd_kwargs):
    nc = _get(mm_dtype)
    in_maps = _prep_inputs(x, weight, mm_dtype)
    res = run_bass_kernel_spmd(nc, in_maps, core_ids=list(range(NCORES)),
                               **spmd_kwargs)
    out = np.empty((B, COUT, OH, OW), dtype=np.float32)
    for c in range(NCORES):
        out[c * BL:(c + 1) * BL] = res.results[c]["y"].reshape(BL, COUT, OH, OW)
    return out, res


def kernel(x, weight):
    out, _ = run(x, weight)
    return out


# revision 9
# speedup vs baseline: 1.1132x; 1.0991x over previous
"""Trainium2 Bass kernel for CropConv (stride-2 3x3 conv, B=32 CIN=COUT=256,
H=W=64 -> 32x32; the crop mask is provably all-ones so output == conv output).

Strategy: data-parallel over batch across 8 NeuronCores (4 images/core).
Host pads each image to 66x66 and splits it into 4 polyphase components
(row/col parity), so each conv tap's moving operand is a fully contiguous
window. Per core the conv is 18 accumulated matmuls per 512-position output
tile: 9 taps x 2 cin-128-chunks, contracting cin on the PE partition dim.
PSUM accumulates fp32. Matmul operands are fp16 (11-bit mantissa; data is
unit-scale so no range issues) giving 1 cycle/row PE throughput with
~3e-4 relative error vs the fp32 reference.
"""

import numpy as np

import concourse.bacc as bacc
import concourse.mybir as mybir
import concourse.tile as tile
from concourse.bass_utils import run_bass_kernel_spmd

B, CIN, COUT, H, W = 32, 256, 256, 64, 64
OH, OW = 32, 32
NCORES = 8
BL = B // NCORES          # images per core
KC = CIN // 128           # cin chunks
MC = COUT // 128          # cout chunks
NT = 2                    # output row-halves per image (16 rows x 32 cols = 512)
RT = OH // NT             # out rows per tile
PR = 17                   # phase rows per half (16 + 1 halo)
PC = 33                   # phase cols
XHALF = 4 * PR * PC       # free size of one x half-tile

TAPS = [(ky, kx) for ky in range(3) for kx in range(3)]

_CACHE = {}


def _build(mm_dtype="f16"):
    io_dt = {
        "bf16": mybir.dt.bfloat16,
        "f16": mybir.dt.float16,
        "f32r": mybir.dt.float32r,
        "f32": mybir.dt.float32,
    }[mm_dtype]
    nc = bacc.Bacc("TRN2", target_bir_lowering=False, debug=False, num_devices=NCORES)
    x = nc.dram_tensor("x", [BL, KC, NT, 128, XHALF], io_dt, kind="ExternalInput")
    w = nc.dram_tensor("w", [MC, 128, 9 * KC * 128], io_dt, kind="ExternalInput")
    y = nc.dram_tensor("y", [BL, MC, 128, OH * OW], mybir.dt.float32,
                       kind="ExternalOutput")

    with tile.TileContext(nc) as tc:
        with (
            tc.tile_pool(name="wpool", bufs=MC) as wpool,
            tc.tile_pool(name="xpool", bufs=BL * KC * NT) as xpool,
            tc.tile_pool(name="opool", bufs=6) as opool,
            tc.tile_pool(name="psum", bufs=8, space="PSUM") as psum_pool,
        ):
            # DMAs emitted in need-order: w[0], then image 0's halves, etc.
            w_sb = {}
            x_sb = {}
            w_sb[0] = wpool.tile([128, 9 * KC * 128], io_dt, tag="wsb", name="wsb0")
            nc.sync.dma_start(w_sb[0][:], w.ap()[0])
            for b in range(BL):
                for nt in range(NT):
                    for kc in range(KC):
                        t = xpool.tile([128, XHALF], io_dt, tag="ximg")
                        nc.sync.dma_start(t[:], x.ap()[b, kc, nt])
                        x_sb[(b, kc, nt)] = t
                if b == 0:
                    w_sb[1] = wpool.tile([128, 9 * KC * 128], io_dt, tag="wsb", name="wsb1")
                    nc.sync.dma_start(w_sb[1][:], w.ap()[1])

            for b in range(BL):
                for mc in range(MC):
                    for nt in range(NT):
                        ps = psum_pool.tile([128, RT * OW], mybir.dt.float32)
                        n_mm = KC * len(TAPS)
                        i_mm = 0
                        for kc in range(KC):
                            xv = x_sb[(b, kc, nt)][:].rearrange(
                                "p (ph r c) -> p ph r c", ph=4, c=PC)
                            for (ky, kx) in TAPS:
                                phase = (ky % 2) * 2 + (kx % 2)
                                r0 = ky // 2
                                c0 = kx // 2
                                rhs = xv[:, phase, r0:r0 + RT, c0:c0 + OW]
                                lhsT = w_sb[mc][:, ((ky * 3 + kx) * KC + kc)
                                                * 128:][:, :128]
                                nc.tensor.matmul(
                                    ps[:], lhsT, rhs,
                                    start=(i_mm == 0), stop=(i_mm == n_mm - 1),
                                )
                                i_mm += 1
                        for h in range(2):
                            ot = opool.tile([128, RT * OW // 2],
                                            mybir.dt.float32, tag="ostage")
                            nc.vector.tensor_copy(
                                ot[:], ps[:, h * 256:(h + 1) * 256])
                            nc.sync.dma_start(
                                y.ap()[b, mc, :,
                                       nt * 512 + h * 256:
                                       nt * 512 + (h + 1) * 256],
                                ot[:],
                            )
    nc.compile()
    return nc


def _get(mm_dtype="f16"):
    if mm_dtype not in _CACHE:
        _CACHE[mm_dtype] = _build(mm_dtype)
    return _CACHE[mm_dtype]


def _np_dt(mm_dtype):
    if mm_dtype == "bf16":
        import ml_dtypes
        return ml_dtypes.bfloat16
    if mm_dtype == "f16":
        return np.float16
    return np.float32


def _prep_inputs(x, weight, mm_dtype="f16"):
    np_dt = _np_dt(mm_dtype)
    # x: [B, CIN, H, W] -> pad to 66x66 (top/left zero) -> 4 polyphase
    # components [pr, pc, 33, 33] -> row-halves with 1-row halo.
    xf = np.asarray(x, dtype=np.float32)
    xp = np.zeros((B, CIN, 66, 66), dtype=np_dt)
    xp[:, :, 1:1 + H, 1:1 + W] = xf
    xph = xp.reshape(B, CIN, 33, 2, 33, 2).transpose(0, 1, 3, 5, 2, 4)
    # xph: [B, CIN, pr, pc, 33, 33]
    halves = np.stack([xph[..., 0:PR, :], xph[..., 33 - PR:33, :]], axis=2)
    # halves: [B, CIN, half, pr, pc, PR, PC]
    xs = halves.reshape(NCORES, BL, KC, 128, NT, XHALF).transpose(0, 1, 2, 4, 3, 5)
    xs = np.ascontiguousarray(xs)  # [NCORES, BL, KC, NT, 128, XHALF]
    # weight: [COUT, CIN, 3, 3] -> [mc, p(cin%128), tap, kc, m(cout%128)]
    wh = np.asarray(weight, dtype=np.float32).transpose(2, 3, 1, 0)  # ky,kx,cin,cout
    wh = wh.reshape(9, KC, 128, MC, 128).transpose(3, 2, 0, 1, 4)
    wh = np.ascontiguousarray(wh.reshape(MC, 128, 9 * KC * 128)).astype(np_dt)
    return [{"x": xs[c], "w": wh} for c in range(NCORES)]


def run(x, weight, mm_dtype="f16", **spmd_kwargs):
    nc = _get(mm_dtype)
    in_maps = _prep_inputs(x, weight, mm_dtype)
    res = run_bass_kernel_spmd(nc, in_maps, core_ids=list(range(NCORES)),
                               **spmd_kwargs)
    out = np.empty((B, COUT, OH, OW), dtype=np.float32)
    for c in range(NCORES):
        out[c * BL:(c + 1) * BL] = res.results[c]["y"].reshape(BL, COUT, OH, OW)
    return out, res


def kernel(x, weight):
    out, _ = run(x, weight)
    return out
